# revision 24
# baseline (speedup 1.0000x reference)
"""Causal self-attention + depthwise-conv + out-proj fused TRN2 kernel (v2).

Model (B=4, T=2048, C=1024, H=16, D=64, conv K=4):
    qkv = x @ W_qkv.T ; causal softmax attention per head ;
    y2 = attn + causal_depthwise_conv(attn) + conv_b ; out = y2 @ W_out.T

Sharding over 8 NeuronCores: core c -> (batch b = c//2, head-group g = c%2).
Each core computes q/k/v for its 8 heads (bf16 matmuls against x[b].T),
bf16 flash-style causal attention in transposed [d, t] layout (exp softmax
without max subtraction - logits are O(1)), the depthwise conv as diagonal
matmuls along the channel partition with the residual folded into the
center tap, then a pairwise AllGather of the 128-channel slab per head
pair and half of the output projection columns.

v2 scheduling changes vs v1 (identical math):
  - interleaved emission: V-proj tiles and the next pair's QK projection
    are emitted between attention q-blocks so the in-order PE fills its
    exp-wait gaps with independent matmul work; the conv for q-block tb
    runs right after that block's normalization instead of after the
    whole pair.
  - conv bias is applied by the DVE during psum evacuation
    (tensor_scalar_add with a per-partition bias column) instead of a
    PE matmul against a ones row.
  - out-proj is split: pass1 contracts the 6 slabs of pairs 0-2 (whose
    AllGathers completed long ago) into a bf16 SBUF accumulator while the
    LAST AllGather is still in flight; pass2 adds only pair 3's two slabs
    (16 small matmuls + DVE adds) behind it.
  - gathered slabs are staged in SBUF once per t-block and reused for all
    4 output column blocks (cuts out-proj DRAM reads 8x vs v1).
  - v_ones psum evacuation is one strided copy per t-tile instead of 8.
"""

import numpy as np
import ml_dtypes

import concourse.bacc as bacc
import concourse.mybir as mybir
import concourse.tile as tile
from concourse.bass_utils import run_bass_kernel_spmd

F32 = mybir.dt.float32
BF16 = mybir.dt.bfloat16

B, T, C, H, D, K = 4, 2048, 1024, 16, 64, 4
HC = H // 2  # heads per core (8)
CC = C // 2  # channels per core (512)
NEG = -30000.0
NCORES = 8
REPLICA_GROUPS = [[0, 1], [2, 3], [4, 5], [6, 7]]
NTB = T // 512  # 512-wide t blocks (4)
NTT = T // 128  # 128-wide t tiles (16)
NCT = C // 128  # 128-wide input-channel tiles (8)
NPAIR = 4  # head pairs per core

_NC_CACHE = {}


def build(debug=False, reps=1, qkv_dt=BF16):
    nc = bacc.Bacc(None, num_devices=NCORES)

    xT_d = nc.dram_tensor("xT", [C, T], qkv_dt, kind="ExternalInput")
    wqk_d = nc.dram_tensor("wqk", [C, 1024], qkv_dt, kind="ExternalInput")
    wv_d = nc.dram_tensor("wv", [C, CC], qkv_dt, kind="ExternalInput")
    wout_d = nc.dram_tensor("wout", [C, CC], BF16, kind="ExternalInput")
    ident_d = nc.dram_tensor("ident", [128, 128], BF16, kind="ExternalInput")
    masks_d = nc.dram_tensor("masks", [128, 512], BF16, kind="ExternalInput")
    convdiag_d = nc.dram_tensor("convdiag", [128, NPAIR * K, 128], BF16,
                                kind="ExternalInput")
    convbias_d = nc.dram_tensor("convbias2", [128, NPAIR], F32,
                                kind="ExternalInput")
    outT_d = nc.dram_tensor("outT", [CC, T], BF16, kind="ExternalOutput")

    with tile.TileContext(nc) as tc:
        with (
            tc.tile_pool(name="consts", bufs=1) as consts,
            tc.tile_pool(name="work", bufs=2) as work,
            tc.tile_pool(name="ps_st", bufs=2, space="PSUM") as ps_st,
            tc.tile_pool(name="ps_att", bufs=1, space="PSUM") as ps_att,
            tc.tile_pool(name="ps_mm", bufs=2, space="PSUM") as ps_mm,
            tc.tile_pool(name="dram", bufs=1, space="DRAM") as dram,
        ):
            # ---------- constants / big loads ----------
            # x slabs spread over 3 DMA queues so the first QK projection
            # can start within a few us; small consts go on the scalar queue.
            xT = consts.tile([128, NCT, T], xT_d.dtype, tag="xT")
            xq = [nc.sync, nc.sync, nc.sync, nc.sync,
                  nc.gpsimd, nc.gpsimd, nc.gpsimd, nc.gpsimd]
            for ct in range(NCT):
                xq[ct].dma_start(xT[:, ct, :],
                                 xT_d[128 * ct: 128 * ct + 128, :])
            ident = consts.tile([128, 128], BF16, tag="ident")
            masks = consts.tile([128, 512], BF16, tag="masks")
            nc.scalar.dma_start(ident[:], ident_d[:])
            nc.scalar.dma_start(masks[:], masks_d[:])
            convdiag = consts.tile([128, NPAIR * K, 128], BF16, tag="convdiag")
            convbias = consts.tile([128, NPAIR], F32, tag="convbias")

            for rep in range(reps):
                _emit_body(nc, tc, consts, work, ps_st, ps_att, ps_mm, dram,
                           locals(), rep)

    nc.compile()
    return nc


def _emit_body(nc, tc, consts, work, ps_st, ps_att, ps_mm, dram, env, rep):
    xT = env["xT"]
    ident = env["ident"]
    masks = env["masks"]
    convdiag = env["convdiag"]
    convbias = env["convbias"]
    wv_d = env["wv_d"]
    wqk_d = env["wqk_d"]
    wout_d = env["wout_d"]
    outT_d = env["outT_d"]
    R = f"r{rep}_"

    # wqk for pair 0 goes on the scalar queue ahead of wv so the first QK
    # projection is not stuck behind x slab loads on the sync queue.
    wqk0 = work.tile([128, NCT, 256], wqk_d.dtype, tag="wqk", bufs=2,
                     name=f"{R}wqk0")
    nc.scalar.dma_start(
        wqk0[:],
        wqk_d[:, 0:256].rearrange("(n p) m -> p n m", p=128),
    )
    # wv shares its slot with wout (wv is dead once V is computed).
    # Its DMA is emitted after the QK projection so the shared DMA device
    # prioritizes the x slabs; same for convdiag/convbias (needed even later).
    wv = consts.tile([128, NCT, CC], wv_d.dtype, tag="w2", name=R + "wv")

    v_ones = consts.tile([128, NTT, HC, 128], BF16, tag="v_ones",
                         name=R + "v_ones")
    nc.vector.memset(v_ones[:, :, :, 0:64], 1.0)

    # out-proj staging: gathered slabs of pairs 0-2, pair 3, accumulator
    ysl = consts.tile([128, NTB, 6, 512], BF16, tag="ysl", name=R + "ysl")
    ysl3 = consts.tile([128, NTB, 2, 512], BF16, tag="ysl3", name=R + "ysl3")
    acc = consts.tile([128, 16, 512], BF16, tag="acc", name=R + "acc")

    # ---------- helpers ----------
    def vproj_tile(tt):
        """V^T for one 128-wide t tile -> v_ones[:, tt, :, 64:128]."""
        vps = ps_mm.tile([128, 512], F32, tag="mm", name=f"{R}vps{tt}")
        for ct in range(NCT):
            nc.tensor.matmul(
                vps[:],
                xT[:, ct, tt * 128: tt * 128 + 128],
                wv[:, ct, :],
                start=(ct == 0),
                stop=(ct == NCT - 1),
            )
        # one strided copy: 8 head-slots x 64 channels
        nc.vector.tensor_copy(v_ones[:, tt, :, 64:128], vps[:])

    def load_wqk(p):
        wqk = work.tile([128, NCT, 256], wqk_d.dtype, tag="wqk", bufs=2,
                        name=f"{R}wqk{p}")
        nc.sync.dma_start(
            wqk[:],
            wqk_d[:, 256 * p: 256 * p + 256].rearrange("(n p) m -> p n m",
                                                       p=128),
        )
        return wqk

    def qkproj_chunk(p, wqk, qT, kT, fs, tb):
        """One [128, 512] strip of q^T (fs=0) or k^T (fs=1) for pair p."""
        ps = ps_mm.tile([128, 512], F32, tag="mm", name=f"{R}qkps{p}_{fs}_{tb}")
        for ct in range(NCT):
            nc.tensor.matmul(
                ps[:],
                wqk[:, ct, 128 * fs: 128 * fs + 128],
                xT[:, ct, 512 * tb: 512 * tb + 512],
                start=(ct == 0),
                stop=(ct == NCT - 1),
            )
        dst = qT if fs == 0 else kT
        nc.vector.tensor_copy(dst[:, 512 * tb: 512 * tb + 512], ps[:])

    def conv_block(p, tb, yt, y2p, y2loc):
        """Depthwise conv + residual + bias for one 512-wide t block."""
        t0 = 512 * tb
        cps = ps_mm.tile([128, 512], F32, tag="mm", name=f"{R}cps{p}_{tb}")
        for lag in range(4):
            j = 3 - lag  # tap index; lag 0 tap has +1 residual
            lo = max(0, lag - t0)
            nc.tensor.matmul(
                cps[:, lo:512],
                convdiag[:, K * p + j, :],
                yt[:, t0 + lo - lag: t0 + 512 - lag],
                start=(lag == 0),
                stop=(lag == 3),
            )
        nc.vector.tensor_scalar_add(
            y2p[:, t0: t0 + 512], cps[:], convbias[:, p: p + 1]
        )
        nc.gpsimd.dma_start(y2loc[:, t0: t0 + 512], y2p[:, t0: t0 + 512])

    # ---------- prologue: QK proj for pair 0, V proj tiles 0-3 ----------
    wqk_tiles = [wqk0]
    qT_tiles = [work.tile([128, T], BF16, tag="qT", bufs=2, name=f"{R}qT0")]
    kT_tiles = [work.tile([128, T], BF16, tag="kT", bufs=2, name=f"{R}kT0")]
    for fs in range(2):
        for tb in range(NTB):
            qkproj_chunk(0, wqk_tiles[0], qT_tiles[0], kT_tiles[0], fs, tb)
    nc.scalar.dma_start(wv[:], wv_d.rearrange("(n p) m -> p n m", p=128))
    if rep == 0:
        nc.scalar.dma_start(convdiag[:], env["convdiag_d"][:])
        nc.scalar.dma_start(convbias[:], env["convbias_d"][:])
    for tt in range(4):
        vproj_tile(tt)

    wout = None
    y2g_tiles = []
    for p in range(NPAIR):
        qT, kT = qT_tiles[p], kT_tiles[p]
        if p < NPAIR - 1:
            wqk_tiles.append(load_wqk(p + 1))
            qT_tiles.append(work.tile([128, T], BF16, tag="qT", bufs=2,
                                      name=f"{R}qT{p + 1}"))
            kT_tiles.append(work.tile([128, T], BF16, tag="kT", bufs=2,
                                      name=f"{R}kT{p + 1}"))
        if p >= 1:
            # stage the previous pair's gathered slabs; emitted AFTER this
            # pair's wqk load so the AllGather wait cannot block it on the
            # in-order SP DMA queue.
            pprev = p - 1
            for s in range(2):
                nc.sync.dma_start(
                    ysl[:, :, 2 * pprev + s, :],
                    y2g_tiles[pprev][128 * s: 128 * s + 128, :].rearrange(
                        "p (tb c) -> p tb c", c=512),
                )

        # filler work units to interleave at the qb boundaries of this pair
        fillers = [[] for _ in range(NTB)]
        if p == 0:
            # v-proj tile tt must be done before AV of q-block qb >= tt//4;
            # schedule tiles 4tt4..4tt4+3 at boundary qb=tt4-1.
            for tt4 in range(1, 4):
                fillers[tt4 - 1].extend(
                    lambda tt=tt: vproj_tile(tt)
                    for tt in range(4 * tt4, 4 * tt4 + 4)
                )
        if p < NPAIR - 1:
            wq = wqk_tiles[p + 1]
            qn, kn = qT_tiles[p + 1], kT_tiles[p + 1]
            chunks = [(fs, tb) for fs in range(2) for tb in range(NTB)]
            for i, (fs, tb) in enumerate(chunks):
                fillers[i % NTB].append(
                    lambda fs=fs, tb=tb: qkproj_chunk(p + 1, wq, qn, kn,
                                                      fs, tb)
                )

        y2loc = dram.tile([128, T], BF16, tag=f"y2loc{p}", name=f"{R}y2loc{p}")
        yt = work.tile([128, T], BF16, tag="yt", bufs=2, name=f"{R}yt{p}")
        y2p = work.tile([128, T], BF16, tag="y2p", bufs=2, name=f"{R}y2p{p}")

        # ---------- attention for the two heads of pair p ----------
        for qb in range(NTB):
            q0 = 512 * qb
            att = [
                ps_att.tile([128, 512], F32, tag=f"att{h}", bufs=1,
                            name=f"{R}att{p}_{qb}_{h}")
                for h in range(2)
            ]
            ngrp = 2 * qb + 2

            def emit_scores(grp, h):
                """Mask prefill + score matmuls + exp for one strip.
                Returns the AV emission thunk (run one group later so the
                exp overlaps the next group's score matmuls on the PE)."""
                hp = 64 * h
                st = ps_st.tile([128, 1024], F32, tag="st", bufs=2,
                                name=f"{R}st{p}_{qb}_{grp}_{h}")
                w0s = []
                for half in range(2):
                    kt = 2 * grp + half
                    w0 = max(0, 128 * (kt - 4 * qb))
                    w0s.append(w0)
                    base = 512 * half
                    if w0 > 0 or kt >= 4 * qb:
                        # diagonal tile: mask prefill
                        i = kt - 4 * qb
                        nc.tensor.matmul(
                            st[:, base + w0: base + w0 + 128],
                            ident[:],
                            masks[:, 128 * i: 128 * i + 128],
                            start=True,
                            stop=False,
                        )
                        sc_start = False
                    else:
                        sc_start = True
                    nc.tensor.matmul(
                        st[:, base + w0: base + 512],
                        kT[hp: hp + 64, 128 * kt: 128 * kt + 128],
                        qT[hp: hp + 64, q0 + w0: q0 + 512],
                        start=sc_start,
                        stop=True,
                    )
                pt = work.tile([128, 1024], BF16, tag="pt", bufs=8,
                               name=f"{R}pt{p}_{qb}_{grp}_{h}")
                nc.scalar.activation(
                    out=pt[:, w0s[0]: 1024],
                    in_=st[:, w0s[0]: 1024],
                    func=mybir.ActivationFunctionType.Exp,
                    scale=0.125,
                )

                def emit_av():
                    for half in range(2):
                        kt = 2 * grp + half
                        w0 = w0s[half]
                        base = 512 * half
                        nc.tensor.matmul(
                            att[h][:, w0:512],
                            v_ones[:, kt, 2 * p + h, :],
                            pt[:, base + w0: base + 512],
                            start=(kt == 0),
                            stop=(kt == 4 * qb + 3),
                        )
                return emit_av

            pending_av = []
            for grp in range(ngrp):
                for h in range(2):
                    av = emit_scores(grp, h)
                    pending_av.append(av)
                    if len(pending_av) > 2:
                        pending_av.pop(0)()
            for av in pending_av:
                av()
            for h in range(2):
                rec = work.tile([64, 512], F32, tag="rec", bufs=2,
                                name=f"{R}rec{p}_{qb}_{h}")
                nc.vector.reciprocal_approx_fast(rec[:], att[h][0:64, :])
                nc.vector.tensor_mul(
                    out=yt[64 * h: 64 * h + 64, q0: q0 + 512],
                    in0=att[h][64:128, :],
                    in1=rec[:],
                )
            # interleaved filler BEFORE the conv so the PE works while the
            # DVE runs the normalization the conv depends on
            for f in fillers[qb]:
                f()
            # conv for this block (uses 3 trailing cols of the prev block)
            conv_block(p, qb, yt, y2p, y2loc)

        if p == 0:
            # V fully projected; the wv slot is free for W_out
            wout = consts.tile([128, NCT, CC], BF16, tag="w2", name=R + "wout")
            nc.sync.dma_start(
                wout[:], wout_d.rearrange("(n p) m -> p n m", p=128)
            )

        # ---------- pairwise AllGather of this 128-channel slab ----------
        y2g = dram.tile([256, T], BF16, tag=f"y2g{p}", name=f"{R}y2g{p}")
        nc.gpsimd.collective_compute(
            "AllGather",
            mybir.AluOpType.bypass,
            replica_groups=REPLICA_GROUPS,
            ins=[y2loc.opt()],
            outs=[y2g.opt()],
        )
        y2g_tiles.append(y2g)
        if p == NPAIR - 2:
            # pair 2's slabs: no later wqk load to hide behind; AG(2)
            # completes early in pair 3, well before pass1 needs it.
            for s in range(2):
                nc.sync.dma_start(
                    ysl[:, :, 2 * p + s, :],
                    y2g[128 * s: 128 * s + 128, :].rearrange(
                        "p (tb c) -> p tb c", c=512),
                )

    # ---------- output projection ----------
    # pass1: contract the 6 slabs of pairs 0-2 (AllGathers already done)
    # while pair 3's AllGather is in flight.
    for tb in range(NTB):
        for ot in range(4):
            ops_ = ps_mm.tile([128, 512], F32, tag="mm",
                              name=f"{R}ops{tb}_{ot}")
            for gs in range(6):
                nc.tensor.matmul(
                    ops_[:],
                    wout[:, gs, 128 * ot: 128 * ot + 128],
                    ysl[:, tb, gs, :],
                    start=(gs == 0),
                    stop=(gs == 5),
                )
            nc.vector.tensor_copy(acc[:, 4 * tb + ot, :], ops_[:])

    # pass2: pair 3's two slabs land; 16 small matmuls + adds + stores
    for s in range(2):
        nc.sync.dma_start(
            ysl3[:, :, s, :],
            y2g_tiles[3][128 * s: 128 * s + 128, :].rearrange(
                "p (tb c) -> p tb c", c=512),
        )
    for tb in range(NTB):
        t0 = 512 * tb
        for ot in range(4):
            ps2 = ps_mm.tile([128, 512], F32, tag="mm",
                             name=f"{R}ps2_{tb}_{ot}")
            for s in range(2):
                nc.tensor.matmul(
                    ps2[:],
                    wout[:, 6 + s, 128 * ot: 128 * ot + 128],
                    ysl3[:, tb, s, :],
                    start=(s == 0),
                    stop=(s == 1),
                )
            osb = work.tile([128, 512], BF16, tag="osb", bufs=4,
                            name=f"{R}osb{tb}_{ot}")
            nc.vector.tensor_add(
                out=osb[:], in0=ps2[:], in1=acc[:, 4 * tb + ot, :]
            )
            oq = nc.gpsimd if ot % 2 == 0 else nc.sync
            oq.dma_start(
                outT_d[128 * ot: 128 * ot + 128, t0: t0 + 512], osb[:]
            )


def _make_masks():
    kp = np.arange(128)[:, None]
    col = np.arange(128)[None, :]
    masks = np.zeros((128, 512), np.float32)
    for i in range(4):
        masks[:, 128 * i: 128 * i + 128] = np.where(kp > col, NEG, 0.0)
    return masks.astype(ml_dtypes.bfloat16)


def prepare_in_maps(x, W_qkv, W_out, conv_w, conv_b, qkv_np=ml_dtypes.bfloat16):
    x = np.asarray(x, np.float32)
    W_qkv = np.asarray(W_qkv, np.float32)
    W_out = np.asarray(W_out, np.float32)
    conv_w = np.asarray(conv_w, np.float32).reshape(C, K)
    conv_b = np.asarray(conv_b, np.float32)

    ident = np.eye(128, dtype=np.float32).astype(ml_dtypes.bfloat16)
    masks = _make_masks()

    # gathered channel order: row r of y2g stack -> global channel
    perm = np.empty(C, np.int64)
    for r in range(C):
        p, parity, within = r // 256, (r % 256) // 128, r % 128
        perm[r] = 512 * parity + 128 * p + within

    in_maps = []
    for core in range(NCORES):
        b, g = core // 2, core % 2
        xT = np.ascontiguousarray(x[b].T)  # [C, T]
        # wqk: cols [256p:256p+128] = q rows of pair p (.T), then k rows
        wqk = np.empty((C, 1024), np.float32)
        for p in range(NPAIR):
            r0 = 64 * (8 * g + 2 * p)
            wqk[:, 256 * p: 256 * p + 128] = W_qkv[r0: r0 + 128, :].T
            wqk[:, 256 * p + 128: 256 * p + 256] = W_qkv[
                1024 + r0: 1024 + r0 + 128, :
            ].T
        wv = np.ascontiguousarray(W_qkv[2048 + CC * g: 2048 + CC * g + CC, :].T)
        # W_out columns for this core's output slice, rows in gathered order
        wout = np.ascontiguousarray(
            W_out[CC * g: CC * g + CC, :].T[perm, :]
        ).astype(ml_dtypes.bfloat16)
        # conv diag matrices for this core's 4 channel tiles x 4 taps
        convdiag = np.zeros((128, NPAIR * K, 128), np.float32)
        idx = np.arange(128)
        for p in range(NPAIR):
            for j in range(K):
                w = conv_w[CC * g + 128 * p: CC * g + 128 * p + 128, j]
                if j == K - 1:
                    w = w + 1.0  # residual folded into the lag-0 tap
                convdiag[idx, K * p + j, idx] = w
        # per-partition bias column per pair: convbias2[within, p]
        convbias2 = np.ascontiguousarray(
            conv_b[CC * g: CC * g + CC].reshape(NPAIR, 128).T
        ).astype(np.float32)
        in_maps.append(
            {
                "xT": xT.astype(qkv_np),
                "wqk": wqk.astype(qkv_np),
                "wv": wv.astype(qkv_np),
                "wout": wout,
                "ident": ident,
                "masks": masks,
                "convdiag": convdiag.astype(ml_dtypes.bfloat16),
                "convbias2": convbias2,
            }
        )
    return in_maps


def assemble_output(results):
    out = np.empty((B, T, C), np.float32)
    for core in range(NCORES):
        b, g = core // 2, core % 2
        outT = np.asarray(results[core]["outT"], np.float32)  # [CC, T]
        out[b, :, CC * g: CC * g + CC] = outT.T
    return out


def kernel(x, W_qkv, W_out, conv_w, conv_b):
    if "nc" not in _NC_CACHE:
        _NC_CACHE["nc"] = build()
    nc = _NC_CACHE["nc"]
    in_maps = prepare_in_maps(x, W_qkv, W_out, conv_w, conv_b)
    res = run_bass_kernel_spmd(nc, in_maps, list(range(NCORES)))
    return assemble_output(res.results)


# revision 42
# speedup vs baseline: 2.8726x; 2.8726x over previous
"""Causal self-attention + depthwise-conv + out-proj fused TRN2 kernel (v2).

Model (B=4, T=2048, C=1024, H=16, D=64, conv K=4):
    qkv = x @ W_qkv.T ; causal softmax attention per head ;
    y2 = attn + causal_depthwise_conv(attn) + conv_b ; out = y2 @ W_out.T

Sharding over 8 NeuronCores: core c -> (batch b = c//2, head-group g = c%2).
Each core computes q/k/v for its 8 heads (bf16 matmuls against x[b].T),
bf16 flash-style causal attention in transposed [d, t] layout (exp softmax
without max subtraction - logits are O(1)), the depthwise conv as diagonal
matmuls along the channel partition with the residual folded into the
center tap, then a pairwise AllGather of the 128-channel slab per head
pair and half of the output projection columns.

v2 scheduling changes vs v1 (identical math):
  - interleaved emission: V-proj tiles and the next pair's QK projection
    are emitted between attention q-blocks so the in-order PE fills its
    exp-wait gaps with independent matmul work; the conv for q-block tb
    runs right after that block's normalization instead of after the
    whole pair.
  - conv bias is applied by the DVE during psum evacuation
    (tensor_scalar_add with a per-partition bias column) instead of a
    PE matmul against a ones row.
  - out-proj is split: pass1 contracts the 6 slabs of pairs 0-2 (whose
    AllGathers completed long ago) into a bf16 SBUF accumulator while the
    LAST AllGather is still in flight; pass2 adds only pair 3's two slabs
    (16 small matmuls + DVE adds) behind it.
  - gathered slabs are staged in SBUF once per t-block and reused for all
    4 output column blocks (cuts out-proj DRAM reads 8x vs v1).
  - v_ones psum evacuation is one strided copy per t-tile instead of 8.
"""

import numpy as np
import ml_dtypes

import concourse.bacc as bacc
import concourse.mybir as mybir
import concourse.tile as tile
from concourse.bass_utils import run_bass_kernel_spmd

F32 = mybir.dt.float32
BF16 = mybir.dt.bfloat16

B, T, C, H, D, K = 4, 2048, 1024, 16, 64, 4
HC = H // 2  # heads per core (8)
CC = C // 2  # channels per core (512)
NEG = -30000.0
NCORES = 8
REPLICA_GROUPS = [[0, 1], [2, 3], [4, 5], [6, 7]]
NTB = T // 512  # 512-wide t blocks (4)
NTT = T // 128  # 128-wide t tiles (16)
NCT = C // 128  # 128-wide input-channel tiles (8)
NPAIR = 4  # head pairs per core

_NC_CACHE = {}


def build(debug=False, reps=1, qkv_dt=BF16, no_cc=False, ag_split=False):
    nc = bacc.Bacc(None, num_devices=NCORES)

    xT_d = nc.dram_tensor("xT", [C, T], qkv_dt, kind="ExternalInput")
    wqk_d = nc.dram_tensor("wqk", [C, 1024], qkv_dt, kind="ExternalInput")
    wv_d = nc.dram_tensor("wv", [C, CC], qkv_dt, kind="ExternalInput")
    wout_d = nc.dram_tensor("wout", [C, CC], BF16, kind="ExternalInput")
    ident_d = nc.dram_tensor("ident", [128, 128], BF16, kind="ExternalInput")
    masks_d = nc.dram_tensor("masks", [128, 512], BF16, kind="ExternalInput")
    convdiag_d = nc.dram_tensor("convdiag", [128, NPAIR * K, 128], BF16,
                                kind="ExternalInput")
    convbias_d = nc.dram_tensor("convbias2", [128, NPAIR], F32,
                                kind="ExternalInput")
    outT_d = nc.dram_tensor("outT", [CC, T], BF16, kind="ExternalOutput")

    with tile.TileContext(nc) as tc:
        with (
            tc.tile_pool(name="consts", bufs=1) as consts,
            tc.tile_pool(name="work", bufs=2) as work,
            tc.tile_pool(name="ps_st", bufs=2, space="PSUM") as ps_st,
            tc.tile_pool(name="ps_att", bufs=1, space="PSUM") as ps_att,
            tc.tile_pool(name="ps_mm", bufs=2, space="PSUM") as ps_mm,
            tc.tile_pool(name="dram", bufs=1, space="DRAM") as dram,
        ):
            # ---------- constants / big loads ----------
            # x slabs spread over 3 DMA queues so the first QK projection
            # can start within a few us; small consts go on the scalar queue.
            xT = consts.tile([128, NCT, T], xT_d.dtype, tag="xT")
            xq = [nc.sync, nc.sync, nc.sync, nc.sync,
                  nc.gpsimd, nc.gpsimd, nc.gpsimd, nc.gpsimd]
            for ct in range(NCT):
                xq[ct].dma_start(xT[:, ct, :],
                                 xT_d[128 * ct: 128 * ct + 128, :])
            ident = consts.tile([128, 128], BF16, tag="ident")
            masks = consts.tile([128, 512], BF16, tag="masks")
            nc.scalar.dma_start(ident[:], ident_d[:])
            nc.scalar.dma_start(masks[:], masks_d[:])
            convdiag = consts.tile([128, NPAIR * K, 128], BF16, tag="convdiag")
            convbias = consts.tile([128, NPAIR], F32, tag="convbias")

            for rep in range(reps):
                _emit_body(nc, tc, consts, work, ps_st, ps_att, ps_mm, dram,
                           locals(), rep, no_cc, ag_split)

    nc.compile()
    return nc


def _emit_body(nc, tc, consts, work, ps_st, ps_att, ps_mm, dram, env, rep,
               no_cc=False, ag_split=False):
    xT = env["xT"]
    ident = env["ident"]
    masks = env["masks"]
    convdiag = env["convdiag"]
    convbias = env["convbias"]
    wv_d = env["wv_d"]
    wqk_d = env["wqk_d"]
    wout_d = env["wout_d"]
    outT_d = env["outT_d"]
    R = f"r{rep}_"

    # wqk for pair 0 goes on the scalar queue ahead of wv so the first QK
    # projection is not stuck behind x slab loads on the sync queue.
    wqk0 = work.tile([128, NCT, 256], wqk_d.dtype, tag="wqk", bufs=2,
                     name=f"{R}wqk0")
    nc.scalar.dma_start(
        wqk0[:],
        wqk_d[:, 0:256].rearrange("(n p) m -> p n m", p=128),
    )
    # wv shares its slot with wout (wv is dead once V is computed).
    # Its DMA is emitted after the QK projection so the shared DMA device
    # prioritizes the x slabs; same for convdiag/convbias (needed even later).
    wv = consts.tile([128, NCT, CC], wv_d.dtype, tag="w2", name=R + "wv")

    v_ones = consts.tile([128, NTT, HC, 128], BF16, tag="v_ones",
                         name=R + "v_ones")
    nc.vector.memset(v_ones[:, :, :, 0:64], 1.0)

    # out-proj staging: gathered slabs of pairs 0-2, pair 3, accumulator
    ysl = consts.tile([128, NTB, 6, 512], BF16, tag="ysl", name=R + "ysl")
    ysl3 = consts.tile([128, NTB, 2, 512], BF16, tag="ysl3", name=R + "ysl3")
    acc = consts.tile([128, 16, 512], BF16, tag="acc", name=R + "acc")

    # ---------- helpers ----------
    def vproj_tile(tt):
        """V^T for one 128-wide t tile -> v_ones[:, tt, :, 64:128]."""
        vps = ps_mm.tile([128, 512], F32, tag="mm", name=f"{R}vps{tt}")
        for ct in range(NCT):
            nc.tensor.matmul(
                vps[:],
                xT[:, ct, tt * 128: tt * 128 + 128],
                wv[:, ct, :],
                start=(ct == 0),
                stop=(ct == NCT - 1),
            )
        # one strided copy: 8 head-slots x 64 channels
        nc.vector.tensor_copy(v_ones[:, tt, :, 64:128], vps[:])

    def load_wqk(p):
        wqk = work.tile([128, NCT, 256], wqk_d.dtype, tag="wqk", bufs=2,
                        name=f"{R}wqk{p}")
        nc.sync.dma_start(
            wqk[:],
            wqk_d[:, 256 * p: 256 * p + 256].rearrange("(n p) m -> p n m",
                                                       p=128),
        )
        return wqk

    def qkproj_chunk(p, wqk, qT, kT, fs, tb):
        """One [128, 512] strip of q^T (fs=0) or k^T (fs=1) for pair p."""
        ps = ps_mm.tile([128, 512], F32, tag="mm", name=f"{R}qkps{p}_{fs}_{tb}")
        for ct in range(NCT):
            nc.tensor.matmul(
                ps[:],
                wqk[:, ct, 128 * fs: 128 * fs + 128],
                xT[:, ct, 512 * tb: 512 * tb + 512],
                start=(ct == 0),
                stop=(ct == NCT - 1),
            )
        dst = qT if fs == 0 else kT
        nc.vector.tensor_copy(dst[:, 512 * tb: 512 * tb + 512], ps[:])

    def conv_block(p, tb, yt, y2p, dst):
        """Depthwise conv + residual + bias for one 512-wide t block."""
        t0 = 512 * tb
        cps = ps_mm.tile([128, 512], F32, tag="mm", name=f"{R}cps{p}_{tb}")
        for lag in range(4):
            j = 3 - lag  # tap index; lag 0 tap has +1 residual
            lo = max(0, lag - t0)
            nc.tensor.matmul(
                cps[:, lo:512],
                convdiag[:, K * p + j, :],
                yt[:, t0 + lo - lag: t0 + 512 - lag],
                start=(lag == 0),
                stop=(lag == 3),
            )
        nc.vector.tensor_scalar_add(
            y2p[:, t0: t0 + 512], cps[:], convbias[:, p: p + 1]
        )
        nc.gpsimd.dma_start(dst, y2p[:, t0: t0 + 512])

    def stage_ysl(pp):
        """DMA pair pp's gathered slabs from DRAM into the ysl staging."""
        if ag_split:
            for tb in range(NTB):
                nc.sync.dma_start(
                    ysl[:, tb, 2 * pp: 2 * pp + 2, :],
                    y2g_tiles[pp][tb][:].rearrange("(s p) c -> p s c", p=128),
                )
        else:
            for s in range(2):
                nc.sync.dma_start(
                    ysl[:, :, 2 * pp + s, :],
                    y2g_tiles[pp][128 * s: 128 * s + 128, :].rearrange(
                        "p (tb c) -> p tb c", c=512),
                )

    # ---------- prologue: QK proj for pair 0, V proj tiles 0-3 ----------
    wqk_tiles = [wqk0]
    qT_tiles = [work.tile([128, T], BF16, tag="qT", bufs=2, name=f"{R}qT0")]
    kT_tiles = [work.tile([128, T], BF16, tag="kT", bufs=2, name=f"{R}kT0")]
    for fs in range(2):
        for tb in range(NTB):
            qkproj_chunk(0, wqk_tiles[0], qT_tiles[0], kT_tiles[0], fs, tb)
    nc.scalar.dma_start(wv[:], wv_d.rearrange("(n p) m -> p n m", p=128))
    if rep == 0:
        nc.scalar.dma_start(convdiag[:], env["convdiag_d"][:])
        nc.scalar.dma_start(convbias[:], env["convbias_d"][:])
    for tt in range(4):
        vproj_tile(tt)

    wout = None
    y2g_tiles = []
    for p in range(NPAIR):
        qT, kT = qT_tiles[p], kT_tiles[p]
        if p < NPAIR - 1:
            wqk_tiles.append(load_wqk(p + 1))
            qT_tiles.append(work.tile([128, T], BF16, tag="qT", bufs=2,
                                      name=f"{R}qT{p + 1}"))
            kT_tiles.append(work.tile([128, T], BF16, tag="kT", bufs=2,
                                      name=f"{R}kT{p + 1}"))
        if p >= 1:
            # stage the previous pair's gathered slabs; emitted AFTER this
            # pair's wqk load so the AllGather wait cannot block it on the
            # in-order SP DMA queue.
            stage_ysl(p - 1)

        # filler work units to interleave at the qb boundaries of this pair
        fillers = [[] for _ in range(NTB)]
        if p == 0:
            # v-proj tile tt must be done before AV of q-block qb >= tt//4;
            # schedule tiles 4tt4..4tt4+3 at boundary qb=tt4-1.
            for tt4 in range(1, 4):
                fillers[tt4 - 1].extend(
                    lambda tt=tt: vproj_tile(tt)
                    for tt in range(4 * tt4, 4 * tt4 + 4)
                )
        if p < NPAIR - 1:
            wq = wqk_tiles[p + 1]
            qn, kn = qT_tiles[p + 1], kT_tiles[p + 1]
            chunks = [(fs, tb) for fs in range(2) for tb in range(NTB)]
            for i, (fs, tb) in enumerate(chunks):
                fillers[i % NTB].append(
                    lambda fs=fs, tb=tb: qkproj_chunk(p + 1, wq, qn, kn,
                                                      fs, tb)
                )

        if ag_split:
            y2loc_t = [dram.tile([128, 512], BF16, tag=f"y2loc{p}_{tb}",
                                 name=f"{R}y2loc{p}_{tb}")
                       for tb in range(NTB)]
            y2g_t = [dram.tile([256, 512], BF16, tag=f"y2g{p}_{tb}",
                               name=f"{R}y2g{p}_{tb}")
                     for tb in range(NTB)]
        else:
            y2loc = dram.tile([128, T], BF16, tag=f"y2loc{p}",
                              name=f"{R}y2loc{p}")
            y2g = dram.tile([256, T], BF16, tag=f"y2g{p}", name=f"{R}y2g{p}")
        yt = work.tile([128, T], BF16, tag="yt", bufs=2, name=f"{R}yt{p}")
        y2p = work.tile([128, T], BF16, tag="y2p", bufs=2, name=f"{R}y2p{p}")

        # ---------- attention for the two heads of pair p ----------
        for qb in range(NTB):
            q0 = 512 * qb
            att = [
                ps_att.tile([128, 512], F32, tag=f"att{h}", bufs=1,
                            name=f"{R}att{p}_{qb}_{h}")
                for h in range(2)
            ]
            ngrp = 2 * qb + 2

            def emit_scores(grp, h):
                """Mask prefill + score matmuls + exp for one strip.
                Returns the AV emission thunk (run one group later so the
                exp overlaps the next group's score matmuls on the PE)."""
                hp = 64 * h
                st = ps_st.tile([128, 1024], F32, tag="st", bufs=2,
                                name=f"{R}st{p}_{qb}_{grp}_{h}")
                w0s = []
                for half in range(2):
                    kt = 2 * grp + half
                    w0 = max(0, 128 * (kt - 4 * qb))
                    w0s.append(w0)
                    base = 512 * half
                    if w0 > 0 or kt >= 4 * qb:
                        # diagonal tile: mask prefill
                        i = kt - 4 * qb
                        nc.tensor.matmul(
                            st[:, base + w0: base + w0 + 128],
                            ident[:],
                            masks[:, 128 * i: 128 * i + 128],
                            start=True,
                            stop=False,
                        )
                        sc_start = False
                    else:
                        sc_start = True
                    nc.tensor.matmul(
                        st[:, base + w0: base + 512],
                        kT[hp: hp + 64, 128 * kt: 128 * kt + 128],
                        qT[hp: hp + 64, q0 + w0: q0 + 512],
                        start=sc_start,
                        stop=True,
                    )
                pt = work.tile([128, 1024], BF16, tag="pt", bufs=8,
                               name=f"{R}pt{p}_{qb}_{grp}_{h}")
                nc.scalar.activation(
                    out=pt[:, w0s[0]: 1024],
                    in_=st[:, w0s[0]: 1024],
                    func=mybir.ActivationFunctionType.Exp,
                    scale=0.125,
                )

                def emit_av():
                    for half in range(2):
                        kt = 2 * grp + half
                        w0 = w0s[half]
                        base = 512 * half
                        nc.tensor.matmul(
                            att[h][:, w0:512],
                            v_ones[:, kt, 2 * p + h, :],
                            pt[:, base + w0: base + 512],
                            start=(kt == 0),
                            stop=(kt == 4 * qb + 3),
                        )
                return emit_av

            pending_av = []
            for grp in range(ngrp):
                for h in range(2):
                    av = emit_scores(grp, h)
                    pending_av.append(av)
                    if len(pending_av) > 2:
                        pending_av.pop(0)()
            for av in pending_av:
                av()
            for h in range(2):
                rec = work.tile([64, 512], F32, tag="rec", bufs=2,
                                name=f"{R}rec{p}_{qb}_{h}")
                nc.vector.reciprocal_approx_fast(rec[:], att[h][0:64, :])
                nc.vector.tensor_mul(
                    out=yt[64 * h: 64 * h + 64, q0: q0 + 512],
                    in0=att[h][64:128, :],
                    in1=rec[:],
                )
            # interleaved filler BEFORE the conv so the PE works while the
            # DVE runs the normalization the conv depends on
            for f in fillers[qb]:
                f()
            # conv for this block (uses 3 trailing cols of the prev block)
            if ag_split:
                conv_block(p, qb, yt, y2p, y2loc_t[qb][:])
                nc.gpsimd.collective_compute(
                    "AllGather",
                    mybir.AluOpType.bypass,
                    replica_groups=REPLICA_GROUPS,
                    ins=[y2loc_t[qb].opt()],
                    outs=[y2g_t[qb].opt()],
                )
            else:
                t0 = 512 * qb
                conv_block(p, qb, yt, y2p, y2loc[:, t0: t0 + 512])

        if p == 0:
            # V fully projected; the wv slot is free for W_out
            wout = consts.tile([128, NCT, CC], BF16, tag="w2", name=R + "wout")
            nc.sync.dma_start(
                wout[:], wout_d.rearrange("(n p) m -> p n m", p=128)
            )

        # ---------- pairwise AllGather of this 128-channel slab ----------
        if ag_split:
            y2g_tiles.append(y2g_t)  # per-block AGs already emitted above
        elif no_cc:
            # timing-only variant: local copies instead of the AllGather
            # (wrong peer data; measures the no-collective critical path)
            for s in range(2):
                nc.gpsimd.dma_start(y2g[128 * s: 128 * s + 128, :], y2loc[:])
            y2g_tiles.append(y2g)
        else:
            nc.gpsimd.collective_compute(
                "AllGather",
                mybir.AluOpType.bypass,
                replica_groups=REPLICA_GROUPS,
                ins=[y2loc.opt()],
                outs=[y2g.opt()],
            )
            y2g_tiles.append(y2g)
        if p == NPAIR - 2:
            # pair 2's slabs: no later wqk load to hide behind; AG(2)
            # completes early in pair 3, well before pass1 needs it.
            stage_ysl(p)

    # ---------- output projection ----------
    # pass1: contract the 6 slabs of pairs 0-2 (AllGathers already done)
    # while pair 3's AllGather is in flight.
    for tb in range(NTB):
        for ot in range(4):
            ops_ = ps_mm.tile([128, 512], F32, tag="mm",
                              name=f"{R}ops{tb}_{ot}")
            for gs in range(6):
                nc.tensor.matmul(
                    ops_[:],
                    wout[:, gs, 128 * ot: 128 * ot + 128],
                    ysl[:, tb, gs, :],
                    start=(gs == 0),
                    stop=(gs == 5),
                )
            nc.vector.tensor_copy(acc[:, 4 * tb + ot, :], ops_[:])

    # pass2: pair 3's two slabs land; 16 small matmuls + adds + stores
    if ag_split:
        for tb in range(NTB):
            nc.sync.dma_start(
                ysl3[:, tb, :, :],
                y2g_tiles[3][tb][:].rearrange("(s p) c -> p s c", p=128),
            )
    else:
        for s in range(2):
            nc.sync.dma_start(
                ysl3[:, :, s, :],
                y2g_tiles[3][128 * s: 128 * s + 128, :].rearrange(
                    "p (tb c) -> p tb c", c=512),
            )
    for tb in range(NTB):
        t0 = 512 * tb
        for ot in range(4):
            ps2 = ps_mm.tile([128, 512], F32, tag="mm",
                             name=f"{R}ps2_{tb}_{ot}")
            for s in range(2):
                nc.tensor.matmul(
                    ps2[:],
                    wout[:, 6 + s, 128 * ot: 128 * ot + 128],
                    ysl3[:, tb, s, :],
                    start=(s == 0),
                    stop=(s == 1),
                )
            osb = work.tile([128, 512], BF16, tag="osb", bufs=4,
                            name=f"{R}osb{tb}_{ot}")
            nc.vector.tensor_add(
                out=osb[:], in0=ps2[:], in1=acc[:, 4 * tb + ot, :]
            )
            oq = nc.gpsimd if ot % 2 == 0 else nc.sync
            oq.dma_start(
                outT_d[128 * ot: 128 * ot + 128, t0: t0 + 512], osb[:]
            )


def _make_masks():
    kp = np.arange(128)[:, None]
    col = np.arange(128)[None, :]
    masks = np.zeros((128, 512), np.float32)
    for i in range(4):
        masks[:, 128 * i: 128 * i + 128] = np.where(kp > col, NEG, 0.0)
    return masks.astype(ml_dtypes.bfloat16)


def prepare_in_maps(x, W_qkv, W_out, conv_w, conv_b, qkv_np=ml_dtypes.bfloat16):
    x = np.asarray(x, np.float32)
    W_qkv = np.asarray(W_qkv, np.float32)
    W_out = np.asarray(W_out, np.float32)
    conv_w = np.asarray(conv_w, np.float32).reshape(C, K)
    conv_b = np.asarray(conv_b, np.float32)

    ident = np.eye(128, dtype=np.float32).astype(ml_dtypes.bfloat16)
    masks = _make_masks()

    # gathered channel order: row r of y2g stack -> global channel
    perm = np.empty(C, np.int64)
    for r in range(C):
        p, parity, within = r // 256, (r % 256) // 128, r % 128
        perm[r] = 512 * parity + 128 * p + within

    in_maps = []
    for core in range(NCORES):
        b, g = core // 2, core % 2
        xT = np.ascontiguousarray(x[b].T)  # [C, T]
        # wqk: cols [256p:256p+128] = q rows of pair p (.T), then k rows
        wqk = np.empty((C, 1024), np.float32)
        for p in range(NPAIR):
            r0 = 64 * (8 * g + 2 * p)
            wqk[:, 256 * p: 256 * p + 128] = W_qkv[r0: r0 + 128, :].T
            wqk[:, 256 * p + 128: 256 * p + 256] = W_qkv[
                1024 + r0: 1024 + r0 + 128, :
            ].T
        wv = np.ascontiguousarray(W_qkv[2048 + CC * g: 2048 + CC * g + CC, :].T)
        # W_out columns for this core's output slice, rows in gathered order
        wout = np.ascontiguousarray(
            W_out[CC * g: CC * g + CC, :].T[perm, :]
        ).astype(ml_dtypes.bfloat16)
        # conv diag matrices for this core's 4 channel tiles x 4 taps
        convdiag = np.zeros((128, NPAIR * K, 128), np.float32)
        idx = np.arange(128)
        for p in range(NPAIR):
            for j in range(K):
                w = conv_w[CC * g + 128 * p: CC * g + 128 * p + 128, j]
                if j == K - 1:
                    w = w + 1.0  # residual folded into the lag-0 tap
                convdiag[idx, K * p + j, idx] = w
        # per-partition bias column per pair: convbias2[within, p]
        convbias2 = np.ascontiguousarray(
            conv_b[CC * g: CC * g + CC].reshape(NPAIR, 128).T
        ).astype(np.float32)
        in_maps.append(
            {
                "xT": xT.astype(qkv_np),
                "wqk": wqk.astype(qkv_np),
                "wv": wv.astype(qkv_np),
                "wout": wout,
                "ident": ident,
                "masks": masks,
                "convdiag": convdiag.astype(ml_dtypes.bfloat16),
                "convbias2": convbias2,
            }
        )
    return in_maps


def assemble_output(results):
    out = np.empty((B, T, C), np.float32)
    for core in range(NCORES):
        b, g = core // 2, core % 2
        outT = np.asarray(results[core]["outT"], np.float32)  # [CC, T]
        out[b, :, CC * g: CC * g + CC] = outT.T
    return out


def kernel(x, W_qkv, W_out, conv_w, conv_b):
    if "nc" not in _NC_CACHE:
        _NC_CACHE["nc"] = build()
    nc = _NC_CACHE["nc"]
    in_maps = prepare_in_maps(x, W_qkv, W_out, conv_w, conv_b)
    res = run_bass_kernel_spmd(nc, in_maps, list(range(NCORES)))
    return assemble_output(res.results)
